# revision 8
# baseline (speedup 1.0000x reference)
"""Quantized SwiGLU FFN (int8 weights/acts, per-row/per-col scales) on 8 trn2 cores.

Sharding: data-parallel over tokens (B*S = 8192 -> 1024 tokens/core).
Weights replicated; no collectives. All matmuls run in bf16 (int8 values are
exact in bf16; fp32 PSUM accumulation of integer dot products is exact below
2^24, which holds for these magnitudes).
"""
import sys
for _p in ('/opt/trn_rl_repo', '/root/.axon_site/_ro/trn_rl_repo'):
    if _p not in sys.path:
        sys.path.append(_p)

import numpy as np
import ml_dtypes

import concourse.bass as bass
import concourse.mybir as mybir
import concourse.tile as tile
from concourse import bacc
from concourse.bass_utils import run_bass_kernel_spmd
from concourse.masks import make_identity

BF16 = mybir.dt.bfloat16
F32 = mybir.dt.float32
I8 = mybir.dt.int8

B, S, D, H = 4, 2048, 4096, 11008
NCORES = 8
TOK = B * S // NCORES          # 1024 tokens per core
P = 128
M_TILES = TOK // P             # 8
K1 = D // P                    # 32 contraction tiles for fc1/fc2
K3 = H // P                    # 86 contraction tiles for fc3
KSUB = 4                       # k-subtiles per weight DMA
N1 = 512
N1_CHUNKS = [(i * N1, min(N1, H - i * N1)) for i in range((H + N1 - 1) // N1)]  # 21x512 + 256
N3 = 512
N3_CHUNKS = D // N3            # 8
QMAX = 127.0
MAGIC = 12582912.0             # 1.5 * 2^23: (x + MAGIC) - MAGIC == rne-round(x) for |x| < 2^22
TINY = 1e-30

_CACHED_NC = None


def _round_to_f32(nc, pool, out_ap, in_ap, inv_ap):
    """out = round(in * inv), computed with the magic-number trick (f32, RNE).

    out_ap may alias in_ap. inv_ap is a [P,1] per-partition scalar.
    """
    t = pool.tile([P, N1], F32, tag="roundtmp")
    w = in_ap.shape[-1]
    nc.vector.tensor_scalar(t[:, :w], in_ap, inv_ap, MAGIC,
                            mybir.AluOpType.mult, mybir.AluOpType.add)
    nc.vector.tensor_scalar(out_ap, t[:, :w], MAGIC, None, mybir.AluOpType.subtract)


def _inv_from_amax(nc, pool, amax_ap, tag):
    """Return [P,1] tile holding QMAX / max(amax, TINY)."""
    t = pool.tile([P, 1], F32, tag=f"{tag}_t")
    nc.vector.tensor_scalar(t[:], amax_ap, TINY, None, mybir.AluOpType.max)
    r = pool.tile([P, 1], F32, tag=f"{tag}_r")
    nc.vector.reciprocal(out=r[:], in_=t[:])
    inv = pool.tile([P, 1], F32, tag=f"{tag}_i")
    nc.vector.tensor_scalar(inv[:], r[:], QMAX, None, mybir.AluOpType.mult)
    return inv


def build_nc():
    nc = bacc.Bacc("TRN2", target_bir_lowering=False, debug=False, num_devices=NCORES)

    xT = nc.dram_tensor("xT", [D, TOK], BF16, kind="ExternalInput")
    w1 = nc.dram_tensor("w1", [D, H], BF16, kind="ExternalInput")
    w2 = nc.dram_tensor("w2", [D, H], BF16, kind="ExternalInput")
    w3 = nc.dram_tensor("w3", [H, D], BF16, kind="ExternalInput")
    sx = nc.dram_tensor("scale_x", [TOK], F32, kind="ExternalInput")
    s_w1 = nc.dram_tensor("s_w1", [H], F32, kind="ExternalInput")
    s_w2 = nc.dram_tensor("s_w2", [H], F32, kind="ExternalInput")
    s_w3 = nc.dram_tensor("s_w3", [D], F32, kind="ExternalInput")
    out = nc.dram_tensor("out", [TOK, D], I8, kind="ExternalOutput")
    scale_out = nc.dram_tensor("scale_out", [TOK], F32, kind="ExternalOutput")

    # DRAM spill buffers. z1/z2 hold raw-dot * s_w (no scale_x: it cancels in
    # every quantization and is folded into the scale chain instead).
    y1buf = nc.dram_tensor("y1buf", [TOK, H], F32)
    y2buf = nc.dram_tensor("y2buf", [TOK, H], F32)
    qmt = nc.dram_tensor("qmt", [K3, P, TOK], BF16)  # q_mul transposed, h-tile-major

    w1v = w1.ap().rearrange("(kb s p) h -> kb p s h", p=P, s=KSUB)
    w2v = w2.ap().rearrange("(kb s p) h -> kb p s h", p=P, s=KSUB)
    w3v = w3.ap().rearrange("(k p) d -> k p d", p=P)
    xTv = xT.ap().rearrange("(k p) t -> p k t", p=P)
    sxv = sx.ap().rearrange("(m p) -> p m", p=P)
    sov = scale_out.ap().rearrange("(m p) -> m p", p=P)

    with tile.TileContext(nc) as tc:
        with tc.tile_pool(name="persist", bufs=1) as persist:
            s1acc = persist.tile([P, M_TILES], F32)
            s2acc = persist.tile([P, M_TILES], F32)
            smt = persist.tile([P, M_TILES], F32)    # true s_mul per m (for scale_out)
            amax3 = persist.tile([P, M_TILES], F32)
            sxt = persist.tile([P, M_TILES], F32)
            ident = persist.tile([P, P], F32)
            nc.vector.memset(s1acc[:], 0.0)
            nc.vector.memset(s2acc[:], 0.0)
            nc.vector.memset(amax3[:], 0.0)
            nc.sync.dma_start(out=sxt[:], in_=sxv)
            make_identity(nc, ident[:])

            # ---------------- Phase 1: fc1 + fc2 ----------------
            with tc.tile_pool(name="xpool", bufs=1) as xp, \
                 tc.tile_pool(name="wstream", bufs=2) as wp, \
                 tc.tile_pool(name="evac1", bufs=4) as ep, \
                 tc.tile_pool(name="swrep", bufs=3) as srp, \
                 tc.tile_pool(name="psum1", bufs=4, space="PSUM") as pp:
                xt = xp.tile([P, K1, TOK], BF16)
                nc.sync.dma_start(out=xt[:], in_=xTv)
                for (noff, nsz) in N1_CHUNKS:
                    for wdram, swdram, acc, ybuf in (
                        (w1v, s_w1, s1acc, y1buf),
                        (w2v, s_w2, s2acc, y2buf),
                    ):
                        swrep = srp.tile([P, N1], F32, tag="swrep")
                        nc.sync.dma_start(
                            out=swrep[:, :nsz],
                            in_=swdram.ap()[None, noff:noff + nsz].to_broadcast([P, nsz]))
                        wt = wp.tile([P, K1, N1], BF16, tag="wt")
                        for kb in range(K1 // KSUB):
                            nc.sync.dma_start(
                                out=wt[:, kb * KSUB:(kb + 1) * KSUB, :nsz],
                                in_=wdram[kb, :, :, noff:noff + nsz])
                        for m in range(M_TILES):
                            ps_t = pp.tile([P, N1], F32, tag="p1")
                            for k in range(K1):
                                nc.tensor.matmul(
                                    ps_t[:, :nsz],
                                    xt[:, k, m * P:(m + 1) * P],
                                    wt[:, k, :nsz],
                                    start=(k == 0), stop=(k == K1 - 1))
                            z = ep.tile([P, N1], F32, tag="z")
                            nc.vector.tensor_tensor(out=z[:, :nsz], in0=ps_t[:, :nsz],
                                                    in1=swrep[:, :nsz],
                                                    op=mybir.AluOpType.mult)
                            tmax = ep.tile([P, 1], F32, tag="tmax")
                            nc.vector.tensor_reduce(out=tmax[:], in_=z[:, :nsz],
                                                    axis=mybir.AxisListType.X,
                                                    op=mybir.AluOpType.max,
                                                    apply_absolute_value=True)
                            nc.vector.tensor_tensor(out=acc[:, m:m + 1], in0=acc[:, m:m + 1],
                                                    in1=tmax[:], op=mybir.AluOpType.max)
                            nc.sync.dma_start(out=ybuf[m * P:(m + 1) * P, noff:noff + nsz],
                                              in_=z[:, :nsz])

            # ---------------- Phases 2+3, interleaved in two m-groups ----------------
            # p2(m 0-3) -> p3(group A) while p2(m 4-7) runs on ACT/DVE -> p3(group B).
            GROUPS = [list(range(0, M_TILES // 2)), list(range(M_TILES // 2, M_TILES))]
            GM = M_TILES // 2
            with tc.tile_pool(name="zbuf", bufs=1) as zp, \
                 tc.tile_pool(name="ystream", bufs=3) as yp, \
                 tc.tile_pool(name="p2tmp", bufs=2) as tp, \
                 tc.tile_pool(name="p2small", bufs=2) as sp, \
                 tc.tile_pool(name="p2stage", bufs=3) as stp, \
                 tc.tile_pool(name="psum2", bufs=2, space="PSUM") as pp2, \
                 tc.tile_pool(name="y3buf", bufs=1) as y3p, \
                 tc.tile_pool(name="qstream", bufs=6) as qsp, \
                 tc.tile_pool(name="w3stream", bufs=6) as w3p, \
                 tc.tile_pool(name="sw3rep", bufs=2) as sr3, \
                 tc.tile_pool(name="evac3", bufs=4) as e3, \
                 tc.tile_pool(name="outp", bufs=2) as op_, \
                 tc.tile_pool(name="psum3", bufs=1, space="PSUM") as pp3:

                def p2_m(m):
                    zb1 = zp.tile([P, H], F32, tag="zb1", name=f"zb1_{m}")

                    inv1 = _inv_from_amax(nc, sp, s1acc[:, m:m + 1], "inv1")
                    inv2 = _inv_from_amax(nc, sp, s2acc[:, m:m + 1], "inv2")
                    s1t = sp.tile([P, 1], F32, tag="s1t")
                    nc.vector.tensor_scalar(s1t[:], s1acc[:, m:m + 1], sxt[:, m:m + 1],
                                            1.0 / QMAX, mybir.AluOpType.mult,
                                            mybir.AluOpType.mult)
                    s2t = sp.tile([P, 1], F32, tag="s2t")
                    nc.vector.tensor_scalar(s2t[:], s2acc[:, m:m + 1], sxt[:, m:m + 1],
                                            1.0 / QMAX, mybir.AluOpType.mult,
                                            mybir.AluOpType.mult)

                    # pass A: zb1 <- f_silu = Silu(round(y1*inv1) * s1t)
                    samax = sp.tile([P, 1], F32, tag="samax")
                    nc.vector.memset(samax[:], 0.0)
                    for (noff, nsz) in N1_CHUNKS:
                        c = zb1[:, noff:noff + nsz]
                        y1c = yp.tile([P, N1], F32, tag="y1c")
                        nc.sync.dma_start(out=y1c[:, :nsz],
                                          in_=y1buf[m * P:(m + 1) * P, noff:noff + nsz])
                        ta = tp.tile([P, N1], F32, tag="ta")
                        nc.scalar.activation(out=ta[:, :nsz], in_=y1c[:, :nsz],
                                             func=mybir.ActivationFunctionType.Copy,
                                             bias=MAGIC, scale=inv1[:])
                        nc.vector.tensor_scalar(y1c[:, :nsz], ta[:, :nsz], MAGIC, None,
                                                mybir.AluOpType.subtract)
                        nc.scalar.activation(out=c, in_=y1c[:, :nsz],
                                             func=mybir.ActivationFunctionType.Silu,
                                             bias=0.0, scale=s1t[:])
                        tmax = tp.tile([P, 1], F32, tag="tmax2")
                        nc.vector.tensor_reduce(out=tmax[:], in_=c,
                                                axis=mybir.AxisListType.X,
                                                op=mybir.AluOpType.max,
                                                apply_absolute_value=True)
                        nc.vector.tensor_tensor(out=samax[:], in0=samax[:], in1=tmax[:],
                                                op=mybir.AluOpType.max)

                    inv_s = _inv_from_amax(nc, sp, samax[:], "invs")
                    ss = sp.tile([P, 1], F32, tag="ss")
                    nc.vector.tensor_scalar(ss[:], samax[:], s2t[:], 1.0 / QMAX,
                                            mybir.AluOpType.mult, mybir.AluOpType.mult)

                    # pass B: zb1 <- prod = (round(f_silu*inv_s)*ss) * round(y2*inv2)
                    mmax = sp.tile([P, 1], F32, tag="mmax")
                    nc.vector.memset(mmax[:], 0.0)
                    for (noff, nsz) in N1_CHUNKS:
                        c1 = zb1[:, noff:noff + nsz]
                        y2c = yp.tile([P, N1], F32, tag="y2c")
                        nc.sync.dma_start(out=y2c[:, :nsz],
                                          in_=y2buf[m * P:(m + 1) * P, noff:noff + nsz])
                        c2 = y2c[:, :nsz]
                        tb = tp.tile([P, N1], F32, tag="tb")
                        nc.scalar.activation(out=tb[:, :nsz], in_=c2,
                                             func=mybir.ActivationFunctionType.Copy,
                                             bias=MAGIC, scale=inv2[:])
                        nc.vector.tensor_scalar(c2, tb[:, :nsz], MAGIC, None,
                                                mybir.AluOpType.subtract)
                        qs = tp.tile([P, N1], F32, tag="qs")
                        tq = tp.tile([P, N1], F32, tag="tq")
                        nc.scalar.activation(out=tq[:, :nsz], in_=c1,
                                             func=mybir.ActivationFunctionType.Copy,
                                             bias=MAGIC, scale=inv_s[:])
                        nc.vector.tensor_scalar(qs[:, :nsz], tq[:, :nsz], MAGIC, None,
                                                mybir.AluOpType.subtract)
                        nc.vector.scalar_tensor_tensor(out=c1, in0=qs[:, :nsz], scalar=ss[:],
                                                       in1=c2, op0=mybir.AluOpType.mult,
                                                       op1=mybir.AluOpType.mult)
                        tmax = tp.tile([P, 1], F32, tag="tmax2")
                        nc.vector.tensor_reduce(out=tmax[:], in_=c1,
                                                axis=mybir.AxisListType.X,
                                                op=mybir.AluOpType.max,
                                                apply_absolute_value=True)
                        nc.vector.tensor_tensor(out=mmax[:], in0=mmax[:], in1=tmax[:],
                                                op=mybir.AluOpType.max)

                    inv_m = _inv_from_amax(nc, sp, mmax[:], "invm")
                    nc.vector.tensor_scalar(smt[:, m:m + 1], mmax[:], 1.0 / QMAX, None,
                                            mybir.AluOpType.mult)

                    # pass C: q_mul = round(prod*inv_m); PE-transpose -> qmt
                    for ci, (noff, nsz) in enumerate(N1_CHUNKS):
                        c1 = zb1[:, noff:noff + nsz]
                        qm = tp.tile([P, N1], F32, tag="qm")
                        tm = tp.tile([P, N1], F32, tag="tm")
                        nc.scalar.activation(out=tm[:, :nsz], in_=c1,
                                             func=mybir.ActivationFunctionType.Copy,
                                             bias=MAGIC, scale=inv_m[:])
                        nc.vector.tensor_scalar(qm[:, :nsz], tm[:, :nsz], MAGIC, None,
                                                mybir.AluOpType.subtract)
                        nblk = nsz // P
                        stage = stp.tile([P, KSUB, P], BF16, tag="stage")
                        pst = pp2.tile([P, KSUB * P], F32, tag="pt")
                        for i in range(nblk):
                            nc.tensor.transpose(pst[:, i * P:(i + 1) * P],
                                                qm[:, i * P:(i + 1) * P], ident[:])
                        nc.scalar.copy(out=stage[:, :nblk, :].rearrange("p a b -> p (a b)"),
                                       in_=pst[:, :nblk * P])
                        nc.sync.dma_start(
                            out=qmt.ap()[ci * (N1 // P): ci * (N1 // P) + nblk, :,
                                         m * P:(m + 1) * P].rearrange("hb p t -> p hb t"),
                            in_=stage[:, :nblk, :])

                def p3_group(g, ms):
                    y3b = y3p.tile([P, GM, D], F32, tag="y3b", name=f"y3b_{g}")
                    for n3 in range(N3_CHUNKS):
                        swrep3 = sr3.tile([P, N3], F32, tag="sw3")
                        nc.sync.dma_start(
                            out=swrep3[:],
                            in_=s_w3.ap()[None, n3 * N3:(n3 + 1) * N3].to_broadcast([P, N3]))
                        ps_tiles = [pp3.tile([P, N3], F32, tag=f"p3_{i}",
                                             name=f"p3_{g}_{n3}_{i}") for i in range(GM)]
                        for k in range(K3):
                            qt = qsp.tile([P, GM * P], BF16, tag="qmk")
                            nc.sync.dma_start(out=qt[:],
                                              in_=qmt.ap()[k, :, g * GM * P:(g + 1) * GM * P])
                            wt3 = w3p.tile([P, N3], BF16, tag="w3t")
                            nc.sync.dma_start(out=wt3[:], in_=w3v[k, :, n3 * N3:(n3 + 1) * N3])
                            for i in range(GM):
                                nc.tensor.matmul(ps_tiles[i][:], qt[:, i * P:(i + 1) * P],
                                                 wt3[:], start=(k == 0), stop=(k == K3 - 1))
                        for i, m in enumerate(ms):
                            nc.vector.tensor_tensor(out=y3b[:, i, n3 * N3:(n3 + 1) * N3],
                                                    in0=ps_tiles[i][:], in1=swrep3[:],
                                                    op=mybir.AluOpType.mult)
                            tmax = e3.tile([P, 1], F32, tag="tmax3")
                            nc.vector.tensor_reduce(out=tmax[:],
                                                    in_=y3b[:, i, n3 * N3:(n3 + 1) * N3],
                                                    axis=mybir.AxisListType.X,
                                                    op=mybir.AluOpType.max,
                                                    apply_absolute_value=True)
                            nc.vector.tensor_tensor(out=amax3[:, m:m + 1],
                                                    in0=amax3[:, m:m + 1], in1=tmax[:],
                                                    op=mybir.AluOpType.max)
                    for i, m in enumerate(ms):
                        inv3 = _inv_from_amax(nc, e3, amax3[:, m:m + 1], "inv3")
                        ost = op_.tile([P, D], I8, tag="ost")
                        for c in range(N3_CHUNKS):
                            t = e3.tile([P, N3], F32, tag="fq")
                            nc.scalar.activation(out=t[:], in_=y3b[:, i, c * N3:(c + 1) * N3],
                                                 func=mybir.ActivationFunctionType.Copy,
                                                 bias=MAGIC, scale=inv3[:])
                            nc.vector.tensor_scalar(ost[:, c * N3:(c + 1) * N3], t[:], MAGIC,
                                                    None, mybir.AluOpType.subtract)
                        nc.sync.dma_start(out=out[m * P:(m + 1) * P, :], in_=ost[:])
                        so = e3.tile([P, 1], F32, tag="so")
                        nc.vector.tensor_scalar(so[:], amax3[:, m:m + 1], smt[:, m:m + 1],
                                                1.0 / QMAX, mybir.AluOpType.mult,
                                                mybir.AluOpType.mult)
                        nc.sync.dma_start(out=sov[m], in_=so[:, 0])

                for m in GROUPS[0]:
                    p2_m(m)
                p3_group(0, GROUPS[0])
                for m in GROUPS[1]:
                    p2_m(m)
                p3_group(1, GROUPS[1])
    nc.compile()
    return nc


def _get_nc():
    global _CACHED_NC
    if _CACHED_NC is None:
        _CACHED_NC = build_nc()
    return _CACHED_NC


def kernel(x, scale_x, w1, s_w1, w2, s_w2, w3, s_w3, _trace=False):
    nc = _get_nc()

    x = np.asarray(x, dtype=np.int8)
    scale_x = np.asarray(scale_x, dtype=np.float32)
    w1b = np.asarray(w1, dtype=np.int8).astype(ml_dtypes.bfloat16)
    w2b = np.asarray(w2, dtype=np.int8).astype(ml_dtypes.bfloat16)
    w3b = np.asarray(w3, dtype=np.int8).astype(ml_dtypes.bfloat16)
    s_w1 = np.ascontiguousarray(np.asarray(s_w1, dtype=np.float32))
    s_w2 = np.ascontiguousarray(np.asarray(s_w2, dtype=np.float32))
    s_w3 = np.ascontiguousarray(np.asarray(s_w3, dtype=np.float32))

    x_flat = x.reshape(B * S, D)
    sx_flat = scale_x.reshape(B * S)

    in_maps = []
    for c in range(NCORES):
        sl = slice(c * TOK, (c + 1) * TOK)
        xT = np.ascontiguousarray(x_flat[sl].T).astype(ml_dtypes.bfloat16)
        in_maps.append({
            "xT": xT,
            "w1": w1b, "w2": w2b, "w3": w3b,
            "scale_x": np.ascontiguousarray(sx_flat[sl]),
            "s_w1": s_w1, "s_w2": s_w2, "s_w3": s_w3,
        })

    res = run_bass_kernel_spmd(nc, in_maps, core_ids=list(range(NCORES)), trace=_trace)

    out = np.empty((B * S, D), dtype=np.int8)
    so = np.empty((B * S,), dtype=np.float32)
    for c in range(NCORES):
        sl = slice(c * TOK, (c + 1) * TOK)
        out[sl] = res.results[c]["out"]
        so[sl] = res.results[c]["scale_out"]
    if _trace:
        kernel.last_exec_time_ns = res.exec_time_ns
        kernel.last_results = res
    return out.reshape(B, S, D), so.reshape(B, S)


# revision 9
# speedup vs baseline: 1.2076x; 1.2076x over previous
"""Quantized SwiGLU FFN (int8 weights/acts, per-row/per-col scales) on 8 trn2 cores.

Sharding: data-parallel over tokens (B*S = 8192 -> 1024 tokens/core).
Weights replicated; no collectives. All matmuls run in bf16 (int8 values are
exact in bf16; fp32 PSUM accumulation of integer dot products is exact below
2^24, which holds for these magnitudes).
"""
import sys
for _p in ('/opt/trn_rl_repo', '/root/.axon_site/_ro/trn_rl_repo'):
    if _p not in sys.path:
        sys.path.append(_p)

import numpy as np
import ml_dtypes

import concourse.bass as bass
import concourse.mybir as mybir
import concourse.tile as tile
from concourse import bacc
from concourse.bass_utils import run_bass_kernel_spmd
from concourse.masks import make_identity

BF16 = mybir.dt.bfloat16
F32 = mybir.dt.float32
I8 = mybir.dt.int8

B, S, D, H = 4, 2048, 4096, 11008
NCORES = 8
TOK = B * S // NCORES          # 1024 tokens per core
P = 128
M_TILES = TOK // P             # 8
K1 = D // P                    # 32 contraction tiles for fc1/fc2
K3 = H // P                    # 86 contraction tiles for fc3
KSUB = 4                       # k-subtiles per weight DMA
N1 = 512
N1_CHUNKS = [(i * N1, min(N1, H - i * N1)) for i in range((H + N1 - 1) // N1)]  # 21x512 + 256
N3 = 512
N3_CHUNKS = D // N3            # 8
QMAX = 127.0
MAGIC = 12582912.0             # 1.5 * 2^23: (x + MAGIC) - MAGIC == rne-round(x) for |x| < 2^22
TINY = 1e-30

_CACHED_NC = None


def _round_to_f32(nc, pool, out_ap, in_ap, inv_ap):
    """out = round(in * inv), computed with the magic-number trick (f32, RNE).

    out_ap may alias in_ap. inv_ap is a [P,1] per-partition scalar.
    """
    t = pool.tile([P, N1], F32, tag="roundtmp")
    w = in_ap.shape[-1]
    nc.vector.tensor_scalar(t[:, :w], in_ap, inv_ap, MAGIC,
                            mybir.AluOpType.mult, mybir.AluOpType.add)
    nc.vector.tensor_scalar(out_ap, t[:, :w], MAGIC, None, mybir.AluOpType.subtract)


def _inv_from_amax(nc, pool, amax_ap, tag):
    """Return [P,1] tile holding QMAX / max(amax, TINY)."""
    t = pool.tile([P, 1], F32, tag=f"{tag}_t")
    nc.vector.tensor_scalar(t[:], amax_ap, TINY, None, mybir.AluOpType.max)
    r = pool.tile([P, 1], F32, tag=f"{tag}_r")
    nc.vector.reciprocal(out=r[:], in_=t[:])
    inv = pool.tile([P, 1], F32, tag=f"{tag}_i")
    nc.vector.tensor_scalar(inv[:], r[:], QMAX, None, mybir.AluOpType.mult)
    return inv


def build_nc():
    nc = bacc.Bacc("TRN2", target_bir_lowering=False, debug=False, num_devices=NCORES)

    xT = nc.dram_tensor("xT", [D, TOK], BF16, kind="ExternalInput")
    w1 = nc.dram_tensor("w1", [D, H], BF16, kind="ExternalInput")
    w2 = nc.dram_tensor("w2", [D, H], BF16, kind="ExternalInput")
    w3 = nc.dram_tensor("w3", [H, D], BF16, kind="ExternalInput")
    sx = nc.dram_tensor("scale_x", [TOK], F32, kind="ExternalInput")
    s_w1 = nc.dram_tensor("s_w1", [H], F32, kind="ExternalInput")
    s_w2 = nc.dram_tensor("s_w2", [H], F32, kind="ExternalInput")
    s_w3 = nc.dram_tensor("s_w3", [D], F32, kind="ExternalInput")
    out = nc.dram_tensor("out", [TOK, D], I8, kind="ExternalOutput")
    scale_out = nc.dram_tensor("scale_out", [TOK], F32, kind="ExternalOutput")

    # DRAM spill buffers. z1/z2 hold raw-dot * s_w (no scale_x: it cancels in
    # every quantization and is folded into the scale chain instead).
    y1buf = nc.dram_tensor("y1buf", [TOK, H], F32)
    y2buf = nc.dram_tensor("y2buf", [TOK, H], F32)
    qmt = nc.dram_tensor("qmt", [K3, P, TOK], BF16)  # q_mul transposed, h-tile-major
    fsbuf = nc.dram_tensor("fsbuf", [TOK, H], F32)  # f_silu rows

    w1v = w1.ap().rearrange("(kb s p) h -> kb p s h", p=P, s=KSUB)
    w2v = w2.ap().rearrange("(kb s p) h -> kb p s h", p=P, s=KSUB)
    w3v = w3.ap().rearrange("(k p) d -> k p d", p=P)
    xTv = xT.ap().rearrange("(k p) t -> p k t", p=P)
    sxv = sx.ap().rearrange("(m p) -> p m", p=P)
    sov = scale_out.ap().rearrange("(m p) -> m p", p=P)

    with tile.TileContext(nc) as tc:
        with tc.tile_pool(name="persist", bufs=1) as persist:
            s1acc = persist.tile([P, M_TILES], F32)
            s2acc = persist.tile([P, M_TILES], F32)
            smt = persist.tile([P, M_TILES], F32)    # true s_mul per m (for scale_out)
            amax3 = persist.tile([P, M_TILES], F32)
            sxt = persist.tile([P, M_TILES], F32)
            ident = persist.tile([P, P], F32)
            nc.vector.memset(s1acc[:], 0.0)
            nc.vector.memset(s2acc[:], 0.0)
            nc.vector.memset(amax3[:], 0.0)
            nc.sync.dma_start(out=sxt[:], in_=sxv)
            make_identity(nc, ident[:])

            # fsbuf holds f_silu rows (f32) produced by pass A during fc2.
            ssacc = persist.tile([P, M_TILES], F32)
            nc.vector.memset(ssacc[:], 0.0)

            # ---------------- Phase 1: fc1, then fc2 with pass-A hidden under it ----------------
            with tc.tile_pool(name="xpool", bufs=1) as xp, \
                 tc.tile_pool(name="wstream", bufs=2) as wp, \
                 tc.tile_pool(name="evac1", bufs=4) as ep, \
                 tc.tile_pool(name="swrep", bufs=3) as srp, \
                 tc.tile_pool(name="pAy", bufs=3) as pay, \
                 tc.tile_pool(name="pAt", bufs=2) as pat, \
                 tc.tile_pool(name="pAs", bufs=2) as pas, \
                 tc.tile_pool(name="psum1", bufs=4, space="PSUM") as pp:
                xt = xp.tile([P, K1, TOK], BF16)
                nc.sync.dma_start(out=xt[:], in_=xTv)

                def gemm12(wdram, swdram, acc, ybuf):
                    for (noff, nsz) in N1_CHUNKS:
                        swrep = srp.tile([P, N1], F32, tag="swrep")
                        nc.sync.dma_start(
                            out=swrep[:, :nsz],
                            in_=swdram.ap()[None, noff:noff + nsz].to_broadcast([P, nsz]))
                        wt = wp.tile([P, K1, N1], BF16, tag="wt")
                        for kb in range(K1 // KSUB):
                            nc.sync.dma_start(
                                out=wt[:, kb * KSUB:(kb + 1) * KSUB, :nsz],
                                in_=wdram[kb, :, :, noff:noff + nsz])
                        for m in range(M_TILES):
                            ps_t = pp.tile([P, N1], F32, tag="p1")
                            for k in range(K1):
                                nc.tensor.matmul(
                                    ps_t[:, :nsz],
                                    xt[:, k, m * P:(m + 1) * P],
                                    wt[:, k, :nsz],
                                    start=(k == 0), stop=(k == K1 - 1))
                            z = ep.tile([P, N1], F32, tag="z")
                            nc.vector.tensor_tensor(out=z[:, :nsz], in0=ps_t[:, :nsz],
                                                    in1=swrep[:, :nsz],
                                                    op=mybir.AluOpType.mult)
                            tmax = ep.tile([P, 1], F32, tag="tmax")
                            nc.vector.tensor_reduce(out=tmax[:], in_=z[:, :nsz],
                                                    axis=mybir.AxisListType.X,
                                                    op=mybir.AluOpType.max,
                                                    apply_absolute_value=True)
                            nc.vector.tensor_tensor(out=acc[:, m:m + 1], in0=acc[:, m:m + 1],
                                                    in1=tmax[:], op=mybir.AluOpType.max)
                            nc.sync.dma_start(out=ybuf[m * P:(m + 1) * P, noff:noff + nsz],
                                              in_=z[:, :nsz])

                gemm12(w1v, s_w1, s1acc, y1buf)   # fc1 first: s1acc complete after this
                gemm12(w2v, s_w2, s2acc, y2buf)   # fc2; pass A below overlaps on ACT/DVE

                # pass A (hidden under fc2): f_silu = Silu(round(z1*inv1) * s1t) -> y1buf
                for m in range(M_TILES):
                    inv1 = _inv_from_amax(nc, pas, s1acc[:, m:m + 1], "inv1")
                    s1t = pas.tile([P, 1], F32, tag="s1t")
                    nc.vector.tensor_scalar(s1t[:], s1acc[:, m:m + 1], sxt[:, m:m + 1],
                                            1.0 / QMAX, mybir.AluOpType.mult,
                                            mybir.AluOpType.mult)
                    samax = pas.tile([P, 1], F32, tag="samax")
                    nc.vector.memset(samax[:], 0.0)
                    for (noff, nsz) in N1_CHUNKS:
                        y1c = pay.tile([P, N1], F32, tag="y1c")
                        nc.sync.dma_start(out=y1c[:, :nsz],
                                          in_=y1buf[m * P:(m + 1) * P, noff:noff + nsz])
                        ta = pat.tile([P, N1], F32, tag="ta")
                        nc.scalar.activation(out=ta[:, :nsz], in_=y1c[:, :nsz],
                                             func=mybir.ActivationFunctionType.Copy,
                                             bias=MAGIC, scale=inv1[:])
                        nc.vector.tensor_scalar(y1c[:, :nsz], ta[:, :nsz], MAGIC, None,
                                                mybir.AluOpType.subtract)
                        fsc = pay.tile([P, N1], F32, tag="fsc")
                        nc.scalar.activation(out=fsc[:, :nsz], in_=y1c[:, :nsz],
                                             func=mybir.ActivationFunctionType.Silu,
                                             bias=0.0, scale=s1t[:])
                        tmax = pat.tile([P, 1], F32, tag="tmaxA")
                        nc.vector.tensor_reduce(out=tmax[:], in_=fsc[:, :nsz],
                                                axis=mybir.AxisListType.X,
                                                op=mybir.AluOpType.max,
                                                apply_absolute_value=True)
                        nc.vector.tensor_tensor(out=ssacc[:, m:m + 1],
                                                in0=ssacc[:, m:m + 1], in1=tmax[:],
                                                op=mybir.AluOpType.max)
                        nc.sync.dma_start(out=fsbuf[m * P:(m + 1) * P, noff:noff + nsz],
                                          in_=fsc[:, :nsz])

            # ---------------- Phase 2: gate + requant + transpose ----------------
            with tc.tile_pool(name="zbuf", bufs=1) as zp, \
                 tc.tile_pool(name="ystream", bufs=4) as yp, \
                 tc.tile_pool(name="p2tmp", bufs=3) as tp, \
                 tc.tile_pool(name="p2small", bufs=2) as sp, \
                 tc.tile_pool(name="p2stage", bufs=3) as stp, \
                 tc.tile_pool(name="psum2", bufs=3, space="PSUM") as pp2:
                for m in range(M_TILES):
                    zb1 = zp.tile([P, H], F32, tag="zb1", name=f"zb1_{m}")

                    inv2 = _inv_from_amax(nc, sp, s2acc[:, m:m + 1], "inv2")
                    s2t = sp.tile([P, 1], F32, tag="s2t")
                    nc.vector.tensor_scalar(s2t[:], s2acc[:, m:m + 1], sxt[:, m:m + 1],
                                            1.0 / QMAX, mybir.AluOpType.mult,
                                            mybir.AluOpType.mult)
                    inv_s = _inv_from_amax(nc, sp, ssacc[:, m:m + 1], "invs")
                    ss = sp.tile([P, 1], F32, tag="ss")
                    nc.vector.tensor_scalar(ss[:], ssacc[:, m:m + 1], s2t[:], 1.0 / QMAX,
                                            mybir.AluOpType.mult, mybir.AluOpType.mult)

                    # pass B: zb1 <- prod = (round(f_silu*inv_s)*ss) * round(z2*inv2)
                    mmax = sp.tile([P, 1], F32, tag="mmax")
                    nc.vector.memset(mmax[:], 0.0)
                    for (noff, nsz) in N1_CHUNKS:
                        c1 = zb1[:, noff:noff + nsz]
                        fsc = yp.tile([P, N1], F32, tag="fscB")
                        nc.sync.dma_start(out=fsc[:, :nsz],
                                          in_=fsbuf[m * P:(m + 1) * P, noff:noff + nsz])
                        y2c = yp.tile([P, N1], F32, tag="y2c")
                        nc.sync.dma_start(out=y2c[:, :nsz],
                                          in_=y2buf[m * P:(m + 1) * P, noff:noff + nsz])
                        c2 = y2c[:, :nsz]
                        tb = tp.tile([P, N1], F32, tag="tb")
                        nc.scalar.activation(out=tb[:, :nsz], in_=c2,
                                             func=mybir.ActivationFunctionType.Copy,
                                             bias=MAGIC, scale=inv2[:])
                        nc.vector.tensor_scalar(c2, tb[:, :nsz], MAGIC, None,
                                                mybir.AluOpType.subtract)
                        qs = tp.tile([P, N1], F32, tag="qs")
                        tq = tp.tile([P, N1], F32, tag="tq")
                        nc.scalar.activation(out=tq[:, :nsz], in_=fsc[:, :nsz],
                                             func=mybir.ActivationFunctionType.Copy,
                                             bias=MAGIC, scale=inv_s[:])
                        nc.vector.tensor_scalar(qs[:, :nsz], tq[:, :nsz], MAGIC, None,
                                                mybir.AluOpType.subtract)
                        nc.vector.scalar_tensor_tensor(out=c1, in0=qs[:, :nsz], scalar=ss[:],
                                                       in1=c2, op0=mybir.AluOpType.mult,
                                                       op1=mybir.AluOpType.mult)
                        tmax = tp.tile([P, 1], F32, tag="tmax2")
                        nc.vector.tensor_reduce(out=tmax[:], in_=c1,
                                                axis=mybir.AxisListType.X,
                                                op=mybir.AluOpType.max,
                                                apply_absolute_value=True)
                        nc.vector.tensor_tensor(out=mmax[:], in0=mmax[:], in1=tmax[:],
                                                op=mybir.AluOpType.max)

                    inv_m = _inv_from_amax(nc, sp, mmax[:], "invm")
                    nc.vector.tensor_scalar(smt[:, m:m + 1], mmax[:], 1.0 / QMAX, None,
                                            mybir.AluOpType.mult)

                    # pass C: q_mul = round(prod*inv_m); PE-transpose -> qmt
                    for ci, (noff, nsz) in enumerate(N1_CHUNKS):
                        c1 = zb1[:, noff:noff + nsz]
                        qm = tp.tile([P, N1], F32, tag="qm")
                        tm = tp.tile([P, N1], F32, tag="tm")
                        nc.scalar.activation(out=tm[:, :nsz], in_=c1,
                                             func=mybir.ActivationFunctionType.Copy,
                                             bias=MAGIC, scale=inv_m[:])
                        nc.vector.tensor_scalar(qm[:, :nsz], tm[:, :nsz], MAGIC, None,
                                                mybir.AluOpType.subtract)
                        nblk = nsz // P
                        stage = stp.tile([P, KSUB, P], BF16, tag="stage")
                        pst = pp2.tile([P, KSUB * P], F32, tag="pt")
                        for i in range(nblk):
                            nc.tensor.transpose(pst[:, i * P:(i + 1) * P],
                                                qm[:, i * P:(i + 1) * P], ident[:])
                        nc.scalar.copy(out=stage[:, :nblk, :].rearrange("p a b -> p (a b)"),
                                       in_=pst[:, :nblk * P])
                        nc.sync.dma_start(
                            out=qmt.ap()[ci * (N1 // P): ci * (N1 // P) + nblk, :,
                                         m * P:(m + 1) * P].rearrange("hb p t -> p hb t"),
                            in_=stage[:, :nblk, :])

            # ---------------- Phase 3: fc3 + final quant ----------------
            with tc.tile_pool(name="y3buf", bufs=1) as y3p, \
                 tc.tile_pool(name="qstream", bufs=6) as qsp, \
                 tc.tile_pool(name="w3stream", bufs=6) as w3p, \
                 tc.tile_pool(name="sw3rep", bufs=2) as sr3, \
                 tc.tile_pool(name="evac3", bufs=4) as e3, \
                 tc.tile_pool(name="outp", bufs=2) as op_, \
                 tc.tile_pool(name="psum3", bufs=1, space="PSUM") as pp3:
                y3b = y3p.tile([P, M_TILES, D], F32)
                for n3 in range(N3_CHUNKS):
                    swrep3 = sr3.tile([P, N3], F32, tag="sw3")
                    nc.sync.dma_start(
                        out=swrep3[:],
                        in_=s_w3.ap()[None, n3 * N3:(n3 + 1) * N3].to_broadcast([P, N3]))
                    ps_tiles = [pp3.tile([P, N3], F32, tag=f"p3_{m}", name=f"p3_{n3}_{m}")
                                for m in range(M_TILES)]
                    for k in range(K3):
                        qt = qsp.tile([P, TOK], BF16, tag="qmk")
                        nc.sync.dma_start(out=qt[:], in_=qmt.ap()[k])
                        wt3 = w3p.tile([P, N3], BF16, tag="w3t")
                        nc.sync.dma_start(out=wt3[:], in_=w3v[k, :, n3 * N3:(n3 + 1) * N3])
                        for m in range(M_TILES):
                            nc.tensor.matmul(ps_tiles[m][:], qt[:, m * P:(m + 1) * P],
                                             wt3[:], start=(k == 0), stop=(k == K3 - 1))
                    for m in range(M_TILES):
                        nc.vector.tensor_tensor(out=y3b[:, m, n3 * N3:(n3 + 1) * N3],
                                                in0=ps_tiles[m][:], in1=swrep3[:],
                                                op=mybir.AluOpType.mult)
                        tmax = e3.tile([P, 1], F32, tag="tmax3")
                        nc.vector.tensor_reduce(out=tmax[:],
                                                in_=y3b[:, m, n3 * N3:(n3 + 1) * N3],
                                                axis=mybir.AxisListType.X,
                                                op=mybir.AluOpType.max,
                                                apply_absolute_value=True)
                        nc.vector.tensor_tensor(out=amax3[:, m:m + 1],
                                                in0=amax3[:, m:m + 1], in1=tmax[:],
                                                op=mybir.AluOpType.max)
                for m in range(M_TILES):
                    inv3 = _inv_from_amax(nc, e3, amax3[:, m:m + 1], "inv3")
                    ost = op_.tile([P, D], I8, tag="ost")
                    for c in range(N3_CHUNKS):
                        t = e3.tile([P, N3], F32, tag="fq")
                        nc.scalar.activation(out=t[:], in_=y3b[:, m, c * N3:(c + 1) * N3],
                                             func=mybir.ActivationFunctionType.Copy,
                                             bias=MAGIC, scale=inv3[:])
                        nc.vector.tensor_scalar(ost[:, c * N3:(c + 1) * N3], t[:], MAGIC,
                                                None, mybir.AluOpType.subtract)
                    nc.sync.dma_start(out=out[m * P:(m + 1) * P, :], in_=ost[:])
                    so = e3.tile([P, 1], F32, tag="so")
                    nc.vector.tensor_scalar(so[:], amax3[:, m:m + 1], smt[:, m:m + 1],
                                            1.0 / QMAX, mybir.AluOpType.mult,
                                            mybir.AluOpType.mult)
                    nc.sync.dma_start(out=sov[m], in_=so[:, 0])
    nc.compile()
    return nc


def _get_nc():
    global _CACHED_NC
    if _CACHED_NC is None:
        _CACHED_NC = build_nc()
    return _CACHED_NC


def kernel(x, scale_x, w1, s_w1, w2, s_w2, w3, s_w3, _trace=False):
    nc = _get_nc()

    x = np.asarray(x, dtype=np.int8)
    scale_x = np.asarray(scale_x, dtype=np.float32)
    w1b = np.asarray(w1, dtype=np.int8).astype(ml_dtypes.bfloat16)
    w2b = np.asarray(w2, dtype=np.int8).astype(ml_dtypes.bfloat16)
    w3b = np.asarray(w3, dtype=np.int8).astype(ml_dtypes.bfloat16)
    s_w1 = np.ascontiguousarray(np.asarray(s_w1, dtype=np.float32))
    s_w2 = np.ascontiguousarray(np.asarray(s_w2, dtype=np.float32))
    s_w3 = np.ascontiguousarray(np.asarray(s_w3, dtype=np.float32))

    x_flat = x.reshape(B * S, D)
    sx_flat = scale_x.reshape(B * S)

    in_maps = []
    for c in range(NCORES):
        sl = slice(c * TOK, (c + 1) * TOK)
        xT = np.ascontiguousarray(x_flat[sl].T).astype(ml_dtypes.bfloat16)
        in_maps.append({
            "xT": xT,
            "w1": w1b, "w2": w2b, "w3": w3b,
            "scale_x": np.ascontiguousarray(sx_flat[sl]),
            "s_w1": s_w1, "s_w2": s_w2, "s_w3": s_w3,
        })

    res = run_bass_kernel_spmd(nc, in_maps, core_ids=list(range(NCORES)), trace=_trace)

    out = np.empty((B * S, D), dtype=np.int8)
    so = np.empty((B * S,), dtype=np.float32)
    for c in range(NCORES):
        sl = slice(c * TOK, (c + 1) * TOK)
        out[sl] = res.results[c]["out"]
        so[sl] = res.results[c]["scale_out"]
    if _trace:
        kernel.last_exec_time_ns = res.exec_time_ns
        kernel.last_results = res
    return out.reshape(B, S, D), so.reshape(B, S)


# revision 10
# speedup vs baseline: 1.2258x; 1.0150x over previous
"""Quantized SwiGLU FFN (int8 weights/acts, per-row/per-col scales) on 8 trn2 cores.

Sharding: data-parallel over tokens (B*S = 8192 -> 1024 tokens/core).
Weights replicated; no collectives. All matmuls run in bf16 (int8 values are
exact in bf16; fp32 PSUM accumulation of integer dot products is exact below
2^24, which holds for these magnitudes).
"""
import sys
for _p in ('/opt/trn_rl_repo', '/root/.axon_site/_ro/trn_rl_repo'):
    if _p not in sys.path:
        sys.path.append(_p)

import numpy as np
import ml_dtypes

import concourse.bass as bass
import concourse.mybir as mybir
import concourse.tile as tile
from concourse import bacc
from concourse.bass_utils import run_bass_kernel_spmd
from concourse.masks import make_identity

BF16 = mybir.dt.bfloat16
F32 = mybir.dt.float32
I8 = mybir.dt.int8

B, S, D, H = 4, 2048, 4096, 11008
NCORES = 8
TOK = B * S // NCORES          # 1024 tokens per core
P = 128
M_TILES = TOK // P             # 8
K1 = D // P                    # 32 contraction tiles for fc1/fc2
K3 = H // P                    # 86 contraction tiles for fc3
KSUB = 4                       # k-subtiles per weight DMA
N1 = 512
N1_CHUNKS = [(i * N1, min(N1, H - i * N1)) for i in range((H + N1 - 1) // N1)]  # 21x512 + 256
N3 = 512
N3_CHUNKS = D // N3            # 8
QMAX = 127.0
MAGIC = 12582912.0             # 1.5 * 2^23: (x + MAGIC) - MAGIC == rne-round(x) for |x| < 2^22
TINY = 1e-30

_CACHED_NC = None


def _round_to_f32(nc, pool, out_ap, in_ap, inv_ap):
    """out = round(in * inv), computed with the magic-number trick (f32, RNE).

    out_ap may alias in_ap. inv_ap is a [P,1] per-partition scalar.
    """
    t = pool.tile([P, N1], F32, tag="roundtmp")
    w = in_ap.shape[-1]
    nc.vector.tensor_scalar(t[:, :w], in_ap, inv_ap, MAGIC,
                            mybir.AluOpType.mult, mybir.AluOpType.add)
    nc.vector.tensor_scalar(out_ap, t[:, :w], MAGIC, None, mybir.AluOpType.subtract)


def _inv_from_amax(nc, pool, amax_ap, tag):
    """Return [P,1] tile holding QMAX / max(amax, TINY)."""
    t = pool.tile([P, 1], F32, tag=f"{tag}_t")
    nc.vector.tensor_scalar(t[:], amax_ap, TINY, None, mybir.AluOpType.max)
    r = pool.tile([P, 1], F32, tag=f"{tag}_r")
    nc.vector.reciprocal(out=r[:], in_=t[:])
    inv = pool.tile([P, 1], F32, tag=f"{tag}_i")
    nc.vector.tensor_scalar(inv[:], r[:], QMAX, None, mybir.AluOpType.mult)
    return inv


def build_nc():
    nc = bacc.Bacc("TRN2", target_bir_lowering=False, debug=False, num_devices=NCORES)

    xT = nc.dram_tensor("xT", [D, TOK], BF16, kind="ExternalInput")
    w1 = nc.dram_tensor("w1", [D, H], BF16, kind="ExternalInput")
    w2 = nc.dram_tensor("w2", [D, H], BF16, kind="ExternalInput")
    w3 = nc.dram_tensor("w3", [H, D], BF16, kind="ExternalInput")
    sx = nc.dram_tensor("scale_x", [TOK], F32, kind="ExternalInput")
    s_w1 = nc.dram_tensor("s_w1", [H], F32, kind="ExternalInput")
    s_w2 = nc.dram_tensor("s_w2", [H], F32, kind="ExternalInput")
    s_w3 = nc.dram_tensor("s_w3", [D], F32, kind="ExternalInput")
    out = nc.dram_tensor("out", [TOK, D], I8, kind="ExternalOutput")
    scale_out = nc.dram_tensor("scale_out", [TOK], F32, kind="ExternalOutput")

    # DRAM spill buffers. z1/z2 hold raw-dot * s_w (no scale_x: it cancels in
    # every quantization and is folded into the scale chain instead).
    y1buf = nc.dram_tensor("y1buf", [TOK, H], F32)
    y2buf = nc.dram_tensor("y2buf", [TOK, H], F32)
    qmt = nc.dram_tensor("qmt", [K3, P, TOK], BF16)  # q_mul transposed, h-tile-major
    fsbuf = nc.dram_tensor("fsbuf", [TOK, H], F32)  # f_silu rows

    w1v = w1.ap().rearrange("(kb s p) h -> kb p s h", p=P, s=KSUB)
    w2v = w2.ap().rearrange("(kb s p) h -> kb p s h", p=P, s=KSUB)
    w3v = w3.ap().rearrange("(k p) d -> k p d", p=P)
    xTv = xT.ap().rearrange("(k p) t -> p k t", p=P)
    sxv = sx.ap().rearrange("(m p) -> p m", p=P)
    sov = scale_out.ap().rearrange("(m p) -> m p", p=P)

    with tile.TileContext(nc) as tc:
        with tc.tile_pool(name="persist", bufs=1) as persist:
            s1acc = persist.tile([P, M_TILES], F32)
            s2acc = persist.tile([P, M_TILES], F32)
            smt = persist.tile([P, M_TILES], F32)    # true s_mul per m (for scale_out)
            amax3 = persist.tile([P, M_TILES], F32)
            sxt = persist.tile([P, M_TILES], F32)
            ident = persist.tile([P, P], F32)
            nc.vector.memset(s1acc[:], 0.0)
            nc.vector.memset(s2acc[:], 0.0)
            nc.vector.memset(amax3[:], 0.0)
            nc.sync.dma_start(out=sxt[:], in_=sxv)
            make_identity(nc, ident[:])

            # fsbuf holds f_silu rows (f32) produced by pass A during fc2.
            ssacc = persist.tile([P, M_TILES], F32)
            nc.vector.memset(ssacc[:], 0.0)

            # ---------------- Phase 1: fc1, then fc2 with pass-A hidden under it ----------------
            with tc.tile_pool(name="xpool", bufs=1) as xp, \
                 tc.tile_pool(name="wstream", bufs=2) as wp, \
                 tc.tile_pool(name="evac1", bufs=4) as ep, \
                 tc.tile_pool(name="swrep", bufs=3) as srp, \
                 tc.tile_pool(name="pAy", bufs=4) as pay, \
                 tc.tile_pool(name="pAt", bufs=2) as pat, \
                 tc.tile_pool(name="pAs", bufs=2) as pas, \
                 tc.tile_pool(name="psum1", bufs=4, space="PSUM") as pp:
                xt = xp.tile([P, K1, TOK], BF16)
                nc.sync.dma_start(out=xt[:], in_=xTv)

                def gemm12(wdram, swdram, acc, ybuf):
                    for (noff, nsz) in N1_CHUNKS:
                        swrep = srp.tile([P, N1], F32, tag="swrep")
                        nc.sync.dma_start(
                            out=swrep[:, :nsz],
                            in_=swdram.ap()[None, noff:noff + nsz].to_broadcast([P, nsz]))
                        wt = wp.tile([P, K1, N1], BF16, tag="wt")
                        for kb in range(K1 // KSUB):
                            nc.sync.dma_start(
                                out=wt[:, kb * KSUB:(kb + 1) * KSUB, :nsz],
                                in_=wdram[kb, :, :, noff:noff + nsz])
                        for m in range(M_TILES):
                            ps_t = pp.tile([P, N1], F32, tag="p1")
                            for k in range(K1):
                                nc.tensor.matmul(
                                    ps_t[:, :nsz],
                                    xt[:, k, m * P:(m + 1) * P],
                                    wt[:, k, :nsz],
                                    start=(k == 0), stop=(k == K1 - 1))
                            z = ep.tile([P, N1], F32, tag="z")
                            nc.vector.tensor_tensor(out=z[:, :nsz], in0=ps_t[:, :nsz],
                                                    in1=swrep[:, :nsz],
                                                    op=mybir.AluOpType.mult)
                            tmax = ep.tile([P, 1], F32, tag="tmax")
                            nc.vector.tensor_reduce(out=tmax[:], in_=z[:, :nsz],
                                                    axis=mybir.AxisListType.X,
                                                    op=mybir.AluOpType.max,
                                                    apply_absolute_value=True)
                            nc.vector.tensor_tensor(out=acc[:, m:m + 1], in0=acc[:, m:m + 1],
                                                    in1=tmax[:], op=mybir.AluOpType.max)
                            nc.sync.dma_start(out=ybuf[m * P:(m + 1) * P, noff:noff + nsz],
                                              in_=z[:, :nsz])

                gemm12(w1v, s_w1, s1acc, y1buf)   # fc1 first: s1acc complete after this
                gemm12(w2v, s_w2, s2acc, y2buf)   # fc2; pass A below overlaps on ACT/DVE

                # pass A (hidden under fc2): f_silu = Silu(round(z1*inv1) * s1t) -> y1buf
                for m in range(M_TILES):
                    inv1 = _inv_from_amax(nc, pas, s1acc[:, m:m + 1], "inv1")
                    s1t = pas.tile([P, 1], F32, tag="s1t")
                    nc.vector.tensor_scalar(s1t[:], s1acc[:, m:m + 1], sxt[:, m:m + 1],
                                            1.0 / QMAX, mybir.AluOpType.mult,
                                            mybir.AluOpType.mult)
                    samax = pas.tile([P, 1], F32, tag="samax")
                    nc.vector.memset(samax[:], 0.0)
                    for (noff, nsz) in N1_CHUNKS:
                        y1c = pay.tile([P, N1], F32, tag="y1c")
                        nc.sync.dma_start(out=y1c[:, :nsz],
                                          in_=y1buf[m * P:(m + 1) * P, noff:noff + nsz])
                        ta = pat.tile([P, N1], F32, tag="ta")
                        nc.scalar.activation(out=ta[:, :nsz], in_=y1c[:, :nsz],
                                             func=mybir.ActivationFunctionType.Copy,
                                             bias=MAGIC, scale=inv1[:])
                        nc.vector.tensor_scalar(y1c[:, :nsz], ta[:, :nsz], MAGIC, None,
                                                mybir.AluOpType.subtract)
                        fsc = pay.tile([P, N1], F32, tag="fsc")
                        nc.scalar.activation(out=fsc[:, :nsz], in_=y1c[:, :nsz],
                                             func=mybir.ActivationFunctionType.Silu,
                                             bias=0.0, scale=s1t[:])
                        tmax = pat.tile([P, 1], F32, tag="tmaxA")
                        nc.vector.tensor_reduce(out=tmax[:], in_=fsc[:, :nsz],
                                                axis=mybir.AxisListType.X,
                                                op=mybir.AluOpType.max,
                                                apply_absolute_value=True)
                        nc.vector.tensor_tensor(out=ssacc[:, m:m + 1],
                                                in0=ssacc[:, m:m + 1], in1=tmax[:],
                                                op=mybir.AluOpType.max)
                        nc.sync.dma_start(out=fsbuf[m * P:(m + 1) * P, noff:noff + nsz],
                                          in_=fsc[:, :nsz])

            # ---------------- Phase 2: gate + requant + transpose ----------------
            with tc.tile_pool(name="zbuf", bufs=1) as zp, \
                 tc.tile_pool(name="ystream", bufs=6) as yp, \
                 tc.tile_pool(name="p2tmp", bufs=3) as tp, \
                 tc.tile_pool(name="p2small", bufs=2) as sp, \
                 tc.tile_pool(name="p2stage", bufs=3) as stp, \
                 tc.tile_pool(name="psum2", bufs=3, space="PSUM") as pp2:
                for m in range(M_TILES):
                    zb1 = zp.tile([P, H], F32, tag="zb1", name=f"zb1_{m}")

                    inv2 = _inv_from_amax(nc, sp, s2acc[:, m:m + 1], "inv2")
                    s2t = sp.tile([P, 1], F32, tag="s2t")
                    nc.vector.tensor_scalar(s2t[:], s2acc[:, m:m + 1], sxt[:, m:m + 1],
                                            1.0 / QMAX, mybir.AluOpType.mult,
                                            mybir.AluOpType.mult)
                    inv_s = _inv_from_amax(nc, sp, ssacc[:, m:m + 1], "invs")
                    ss = sp.tile([P, 1], F32, tag="ss")
                    nc.vector.tensor_scalar(ss[:], ssacc[:, m:m + 1], s2t[:], 1.0 / QMAX,
                                            mybir.AluOpType.mult, mybir.AluOpType.mult)

                    # pass B: zb1 <- prod = (round(f_silu*inv_s)*ss) * round(z2*inv2)
                    mmax = sp.tile([P, 1], F32, tag="mmax")
                    nc.vector.memset(mmax[:], 0.0)
                    for (noff, nsz) in N1_CHUNKS:
                        c1 = zb1[:, noff:noff + nsz]
                        fsc = yp.tile([P, N1], F32, tag="fscB")
                        nc.sync.dma_start(out=fsc[:, :nsz],
                                          in_=fsbuf[m * P:(m + 1) * P, noff:noff + nsz])
                        y2c = yp.tile([P, N1], F32, tag="y2c")
                        nc.sync.dma_start(out=y2c[:, :nsz],
                                          in_=y2buf[m * P:(m + 1) * P, noff:noff + nsz])
                        c2 = y2c[:, :nsz]
                        tb = tp.tile([P, N1], F32, tag="tb")
                        nc.scalar.activation(out=tb[:, :nsz], in_=c2,
                                             func=mybir.ActivationFunctionType.Copy,
                                             bias=MAGIC, scale=inv2[:])
                        nc.vector.tensor_scalar(c2, tb[:, :nsz], MAGIC, None,
                                                mybir.AluOpType.subtract)
                        qs = tp.tile([P, N1], F32, tag="qs")
                        tq = tp.tile([P, N1], F32, tag="tq")
                        nc.scalar.activation(out=tq[:, :nsz], in_=fsc[:, :nsz],
                                             func=mybir.ActivationFunctionType.Copy,
                                             bias=MAGIC, scale=inv_s[:])
                        nc.vector.tensor_scalar(qs[:, :nsz], tq[:, :nsz], MAGIC, None,
                                                mybir.AluOpType.subtract)
                        nc.vector.scalar_tensor_tensor(out=c1, in0=qs[:, :nsz], scalar=ss[:],
                                                       in1=c2, op0=mybir.AluOpType.mult,
                                                       op1=mybir.AluOpType.mult)
                        tmax = tp.tile([P, 1], F32, tag="tmax2")
                        nc.vector.tensor_reduce(out=tmax[:], in_=c1,
                                                axis=mybir.AxisListType.X,
                                                op=mybir.AluOpType.max,
                                                apply_absolute_value=True)
                        nc.vector.tensor_tensor(out=mmax[:], in0=mmax[:], in1=tmax[:],
                                                op=mybir.AluOpType.max)

                    inv_m = _inv_from_amax(nc, sp, mmax[:], "invm")
                    nc.vector.tensor_scalar(smt[:, m:m + 1], mmax[:], 1.0 / QMAX, None,
                                            mybir.AluOpType.mult)

                    # pass C: q_mul = round(prod*inv_m); PE-transpose -> qmt
                    for ci, (noff, nsz) in enumerate(N1_CHUNKS):
                        c1 = zb1[:, noff:noff + nsz]
                        qm = tp.tile([P, N1], F32, tag="qm")
                        tm = tp.tile([P, N1], F32, tag="tm")
                        nc.scalar.activation(out=tm[:, :nsz], in_=c1,
                                             func=mybir.ActivationFunctionType.Copy,
                                             bias=MAGIC, scale=inv_m[:])
                        nc.vector.tensor_scalar(qm[:, :nsz], tm[:, :nsz], MAGIC, None,
                                                mybir.AluOpType.subtract)
                        nblk = nsz // P
                        stage = stp.tile([P, KSUB, P], BF16, tag="stage")
                        pst = pp2.tile([P, KSUB * P], F32, tag="pt")
                        for i in range(nblk):
                            nc.tensor.transpose(pst[:, i * P:(i + 1) * P],
                                                qm[:, i * P:(i + 1) * P], ident[:])
                        nc.scalar.copy(out=stage[:, :nblk, :].rearrange("p a b -> p (a b)"),
                                       in_=pst[:, :nblk * P])
                        nc.sync.dma_start(
                            out=qmt.ap()[ci * (N1 // P): ci * (N1 // P) + nblk, :,
                                         m * P:(m + 1) * P].rearrange("hb p t -> p hb t"),
                            in_=stage[:, :nblk, :])

            # ---------------- Phase 3: fc3 + final quant ----------------
            with tc.tile_pool(name="y3buf", bufs=1) as y3p, \
                 tc.tile_pool(name="qstream", bufs=10) as qsp, \
                 tc.tile_pool(name="w3stream", bufs=10) as w3p, \
                 tc.tile_pool(name="sw3rep", bufs=2) as sr3, \
                 tc.tile_pool(name="evac3", bufs=4) as e3, \
                 tc.tile_pool(name="outp", bufs=2) as op_, \
                 tc.tile_pool(name="psum3", bufs=1, space="PSUM") as pp3:
                y3b = y3p.tile([P, M_TILES, D], F32)
                for n3 in range(N3_CHUNKS):
                    swrep3 = sr3.tile([P, N3], F32, tag="sw3")
                    nc.sync.dma_start(
                        out=swrep3[:],
                        in_=s_w3.ap()[None, n3 * N3:(n3 + 1) * N3].to_broadcast([P, N3]))
                    ps_tiles = [pp3.tile([P, N3], F32, tag=f"p3_{m}", name=f"p3_{n3}_{m}")
                                for m in range(M_TILES)]
                    for k in range(K3):
                        qt = qsp.tile([P, TOK], BF16, tag="qmk")
                        nc.sync.dma_start(out=qt[:], in_=qmt.ap()[k])
                        wt3 = w3p.tile([P, N3], BF16, tag="w3t")
                        nc.sync.dma_start(out=wt3[:], in_=w3v[k, :, n3 * N3:(n3 + 1) * N3])
                        for m in range(M_TILES):
                            nc.tensor.matmul(ps_tiles[m][:], qt[:, m * P:(m + 1) * P],
                                             wt3[:], start=(k == 0), stop=(k == K3 - 1))
                    for m in range(M_TILES):
                        nc.vector.tensor_tensor(out=y3b[:, m, n3 * N3:(n3 + 1) * N3],
                                                in0=ps_tiles[m][:], in1=swrep3[:],
                                                op=mybir.AluOpType.mult)
                        tmax = e3.tile([P, 1], F32, tag="tmax3")
                        nc.vector.tensor_reduce(out=tmax[:],
                                                in_=y3b[:, m, n3 * N3:(n3 + 1) * N3],
                                                axis=mybir.AxisListType.X,
                                                op=mybir.AluOpType.max,
                                                apply_absolute_value=True)
                        nc.vector.tensor_tensor(out=amax3[:, m:m + 1],
                                                in0=amax3[:, m:m + 1], in1=tmax[:],
                                                op=mybir.AluOpType.max)
                for m in range(M_TILES):
                    inv3 = _inv_from_amax(nc, e3, amax3[:, m:m + 1], "inv3")
                    ost = op_.tile([P, D], I8, tag="ost")
                    for c in range(N3_CHUNKS):
                        t = e3.tile([P, N3], F32, tag="fq")
                        nc.scalar.activation(out=t[:], in_=y3b[:, m, c * N3:(c + 1) * N3],
                                             func=mybir.ActivationFunctionType.Copy,
                                             bias=MAGIC, scale=inv3[:])
                        nc.vector.tensor_scalar(ost[:, c * N3:(c + 1) * N3], t[:], MAGIC,
                                                None, mybir.AluOpType.subtract)
                    nc.sync.dma_start(out=out[m * P:(m + 1) * P, :], in_=ost[:])
                    so = e3.tile([P, 1], F32, tag="so")
                    nc.vector.tensor_scalar(so[:], amax3[:, m:m + 1], smt[:, m:m + 1],
                                            1.0 / QMAX, mybir.AluOpType.mult,
                                            mybir.AluOpType.mult)
                    nc.sync.dma_start(out=sov[m], in_=so[:, 0])
    nc.compile()
    return nc


def _get_nc():
    global _CACHED_NC
    if _CACHED_NC is None:
        _CACHED_NC = build_nc()
    return _CACHED_NC


def kernel(x, scale_x, w1, s_w1, w2, s_w2, w3, s_w3, _trace=False):
    nc = _get_nc()

    x = np.asarray(x, dtype=np.int8)
    scale_x = np.asarray(scale_x, dtype=np.float32)
    w1b = np.asarray(w1, dtype=np.int8).astype(ml_dtypes.bfloat16)
    w2b = np.asarray(w2, dtype=np.int8).astype(ml_dtypes.bfloat16)
    w3b = np.asarray(w3, dtype=np.int8).astype(ml_dtypes.bfloat16)
    s_w1 = np.ascontiguousarray(np.asarray(s_w1, dtype=np.float32))
    s_w2 = np.ascontiguousarray(np.asarray(s_w2, dtype=np.float32))
    s_w3 = np.ascontiguousarray(np.asarray(s_w3, dtype=np.float32))

    x_flat = x.reshape(B * S, D)
    sx_flat = scale_x.reshape(B * S)

    in_maps = []
    for c in range(NCORES):
        sl = slice(c * TOK, (c + 1) * TOK)
        xT = np.ascontiguousarray(x_flat[sl].T).astype(ml_dtypes.bfloat16)
        in_maps.append({
            "xT": xT,
            "w1": w1b, "w2": w2b, "w3": w3b,
            "scale_x": np.ascontiguousarray(sx_flat[sl]),
            "s_w1": s_w1, "s_w2": s_w2, "s_w3": s_w3,
        })

    res = run_bass_kernel_spmd(nc, in_maps, core_ids=list(range(NCORES)), trace=_trace)

    out = np.empty((B * S, D), dtype=np.int8)
    so = np.empty((B * S,), dtype=np.float32)
    for c in range(NCORES):
        sl = slice(c * TOK, (c + 1) * TOK)
        out[sl] = res.results[c]["out"]
        so[sl] = res.results[c]["scale_out"]
    if _trace:
        kernel.last_exec_time_ns = res.exec_time_ns
        kernel.last_results = res
    return out.reshape(B, S, D), so.reshape(B, S)


# revision 11
# speedup vs baseline: 1.2355x; 1.0080x over previous
"""Quantized SwiGLU FFN (int8 weights/acts, per-row/per-col scales) on 8 trn2 cores.

Sharding: data-parallel over tokens (B*S = 8192 -> 1024 tokens/core).
Weights replicated; no collectives. All matmuls run in bf16 (int8 values are
exact in bf16; fp32 PSUM accumulation of integer dot products is exact below
2^24, which holds for these magnitudes).
"""
import sys
for _p in ('/opt/trn_rl_repo', '/root/.axon_site/_ro/trn_rl_repo'):
    if _p not in sys.path:
        sys.path.append(_p)

import numpy as np
import ml_dtypes

import concourse.bass as bass
import concourse.mybir as mybir
import concourse.tile as tile
from concourse import bacc
from concourse.bass_utils import run_bass_kernel_spmd
from concourse.masks import make_identity

BF16 = mybir.dt.bfloat16
F32 = mybir.dt.float32
I8 = mybir.dt.int8

B, S, D, H = 4, 2048, 4096, 11008
NCORES = 8
TOK = B * S // NCORES          # 1024 tokens per core
P = 128
M_TILES = TOK // P             # 8
K1 = D // P                    # 32 contraction tiles for fc1/fc2
K3 = H // P                    # 86 contraction tiles for fc3
KSUB = 4                       # k-subtiles per weight DMA
N1 = 512
N1_CHUNKS = [(i * N1, min(N1, H - i * N1)) for i in range((H + N1 - 1) // N1)]  # 21x512 + 256
N3 = 512
N3_CHUNKS = D // N3            # 8
QMAX = 127.0
MAGIC = 12582912.0             # 1.5 * 2^23: (x + MAGIC) - MAGIC == rne-round(x) for |x| < 2^22
TINY = 1e-30

_CACHED_NC = None


def _round_to_f32(nc, pool, out_ap, in_ap, inv_ap):
    """out = round(in * inv), computed with the magic-number trick (f32, RNE).

    out_ap may alias in_ap. inv_ap is a [P,1] per-partition scalar.
    """
    t = pool.tile([P, N1], F32, tag="roundtmp")
    w = in_ap.shape[-1]
    nc.vector.tensor_scalar(t[:, :w], in_ap, inv_ap, MAGIC,
                            mybir.AluOpType.mult, mybir.AluOpType.add)
    nc.vector.tensor_scalar(out_ap, t[:, :w], MAGIC, None, mybir.AluOpType.subtract)


def _inv_from_amax(nc, pool, amax_ap, tag):
    """Return [P,1] tile holding QMAX / max(amax, TINY)."""
    t = pool.tile([P, 1], F32, tag=f"{tag}_t")
    nc.vector.tensor_scalar(t[:], amax_ap, TINY, None, mybir.AluOpType.max)
    r = pool.tile([P, 1], F32, tag=f"{tag}_r")
    nc.vector.reciprocal(out=r[:], in_=t[:])
    inv = pool.tile([P, 1], F32, tag=f"{tag}_i")
    nc.vector.tensor_scalar(inv[:], r[:], QMAX, None, mybir.AluOpType.mult)
    return inv


def build_nc():
    nc = bacc.Bacc("TRN2", target_bir_lowering=False, debug=False, num_devices=NCORES)

    xT = nc.dram_tensor("xT", [D, TOK], BF16, kind="ExternalInput")
    w1 = nc.dram_tensor("w1", [D, H], BF16, kind="ExternalInput")
    w2 = nc.dram_tensor("w2", [D, H], BF16, kind="ExternalInput")
    w3 = nc.dram_tensor("w3", [H, D], BF16, kind="ExternalInput")
    sx = nc.dram_tensor("scale_x", [TOK], F32, kind="ExternalInput")
    s_w1 = nc.dram_tensor("s_w1", [H], F32, kind="ExternalInput")
    s_w2 = nc.dram_tensor("s_w2", [H], F32, kind="ExternalInput")
    s_w3 = nc.dram_tensor("s_w3", [D], F32, kind="ExternalInput")
    out = nc.dram_tensor("out", [TOK, D], I8, kind="ExternalOutput")
    scale_out = nc.dram_tensor("scale_out", [TOK], F32, kind="ExternalOutput")

    # DRAM spill buffers. z1/z2 hold raw-dot * s_w (no scale_x: it cancels in
    # every quantization and is folded into the scale chain instead).
    y1buf = nc.dram_tensor("y1buf", [TOK, H], F32)
    y2buf = nc.dram_tensor("y2buf", [TOK, H], F32)
    qmt = nc.dram_tensor("qmt", [K3, P, TOK], BF16)  # q_mul transposed, h-tile-major
    fsbuf = nc.dram_tensor("fsbuf", [TOK, H], F32)  # f_silu rows

    w1v = w1.ap().rearrange("(kb s p) h -> kb p s h", p=P, s=KSUB)
    w2v = w2.ap().rearrange("(kb s p) h -> kb p s h", p=P, s=KSUB)
    w3v = w3.ap().rearrange("(k p) d -> k p d", p=P)
    xTv = xT.ap().rearrange("(k p) t -> p k t", p=P)
    sxv = sx.ap().rearrange("(m p) -> p m", p=P)
    sov = scale_out.ap().rearrange("(m p) -> m p", p=P)

    with tile.TileContext(nc) as tc:
        with tc.tile_pool(name="persist", bufs=1) as persist:
            s1acc = persist.tile([P, M_TILES], F32)
            s2acc = persist.tile([P, M_TILES], F32)
            smt = persist.tile([P, M_TILES], F32)    # true s_mul per m (for scale_out)
            amax3 = persist.tile([P, M_TILES], F32)
            sxt = persist.tile([P, M_TILES], F32)
            ident = persist.tile([P, P], F32)
            nc.vector.memset(s1acc[:], 0.0)
            nc.vector.memset(s2acc[:], 0.0)
            nc.vector.memset(amax3[:], 0.0)
            nc.sync.dma_start(out=sxt[:], in_=sxv)
            make_identity(nc, ident[:])

            # fsbuf holds f_silu rows (f32) produced by pass A during fc2.
            ssacc = persist.tile([P, M_TILES], F32)
            nc.vector.memset(ssacc[:], 0.0)

            # ---------------- Phase 1: fc1, then fc2 with pass-A hidden under it ----------------
            with tc.tile_pool(name="xpool", bufs=1) as xp, \
                 tc.tile_pool(name="wstream", bufs=2) as wp, \
                 tc.tile_pool(name="evac1", bufs=4) as ep, \
                 tc.tile_pool(name="swrep", bufs=3) as srp, \
                 tc.tile_pool(name="pAy", bufs=4) as pay, \
                 tc.tile_pool(name="pAt", bufs=2) as pat, \
                 tc.tile_pool(name="pAs", bufs=2) as pas, \
                 tc.tile_pool(name="psum1", bufs=4, space="PSUM") as pp:
                xt = xp.tile([P, K1, TOK], BF16)
                nc.sync.dma_start(out=xt[:], in_=xTv)

                def gemm12(wdram, swdram, acc, ybuf):
                    for (noff, nsz) in N1_CHUNKS:
                        swrep = srp.tile([P, N1], F32, tag="swrep")
                        nc.sync.dma_start(
                            out=swrep[:, :nsz],
                            in_=swdram.ap()[None, noff:noff + nsz].to_broadcast([P, nsz]))
                        wt = wp.tile([P, K1, N1], BF16, tag="wt")
                        for kb in range(K1 // KSUB):
                            nc.sync.dma_start(
                                out=wt[:, kb * KSUB:(kb + 1) * KSUB, :nsz],
                                in_=wdram[kb, :, :, noff:noff + nsz])
                        for m in range(M_TILES):
                            ps_t = pp.tile([P, N1], F32, tag="p1")
                            for k in range(K1):
                                nc.tensor.matmul(
                                    ps_t[:, :nsz],
                                    xt[:, k, m * P:(m + 1) * P],
                                    wt[:, k, :nsz],
                                    start=(k == 0), stop=(k == K1 - 1))
                            z = ep.tile([P, N1], F32, tag="z")
                            nc.vector.tensor_tensor(out=z[:, :nsz], in0=ps_t[:, :nsz],
                                                    in1=swrep[:, :nsz],
                                                    op=mybir.AluOpType.mult)
                            tmax = ep.tile([P, 1], F32, tag="tmax")
                            nc.vector.tensor_reduce(out=tmax[:], in_=z[:, :nsz],
                                                    axis=mybir.AxisListType.X,
                                                    op=mybir.AluOpType.max,
                                                    apply_absolute_value=True)
                            nc.vector.tensor_tensor(out=acc[:, m:m + 1], in0=acc[:, m:m + 1],
                                                    in1=tmax[:], op=mybir.AluOpType.max)
                            nc.sync.dma_start(out=ybuf[m * P:(m + 1) * P, noff:noff + nsz],
                                              in_=z[:, :nsz])

                gemm12(w1v, s_w1, s1acc, y1buf)   # fc1 first: s1acc complete after this
                gemm12(w2v, s_w2, s2acc, y2buf)   # fc2; pass A below overlaps on ACT/DVE

                # pass A (hidden under fc2): f_silu = Silu(round(z1*inv1) * s1t) -> y1buf
                for m in range(M_TILES):
                    inv1 = _inv_from_amax(nc, pas, s1acc[:, m:m + 1], "inv1")
                    s1t = pas.tile([P, 1], F32, tag="s1t")
                    nc.vector.tensor_scalar(s1t[:], s1acc[:, m:m + 1], sxt[:, m:m + 1],
                                            1.0 / QMAX, mybir.AluOpType.mult,
                                            mybir.AluOpType.mult)
                    samax = pas.tile([P, 1], F32, tag="samax")
                    nc.vector.memset(samax[:], 0.0)
                    for (noff, nsz) in N1_CHUNKS:
                        y1c = pay.tile([P, N1], F32, tag="y1c")
                        nc.sync.dma_start(out=y1c[:, :nsz],
                                          in_=y1buf[m * P:(m + 1) * P, noff:noff + nsz])
                        ta = pat.tile([P, N1], F32, tag="ta")
                        nc.scalar.activation(out=ta[:, :nsz], in_=y1c[:, :nsz],
                                             func=mybir.ActivationFunctionType.Copy,
                                             bias=MAGIC, scale=inv1[:])
                        nc.vector.tensor_scalar(y1c[:, :nsz], ta[:, :nsz], MAGIC, None,
                                                mybir.AluOpType.subtract)
                        fsc = pay.tile([P, N1], F32, tag="fsc")
                        nc.scalar.activation(out=fsc[:, :nsz], in_=y1c[:, :nsz],
                                             func=mybir.ActivationFunctionType.Silu,
                                             bias=0.0, scale=s1t[:])
                        tmax = pat.tile([P, 1], F32, tag="tmaxA")
                        nc.vector.tensor_reduce(out=tmax[:], in_=fsc[:, :nsz],
                                                axis=mybir.AxisListType.X,
                                                op=mybir.AluOpType.max,
                                                apply_absolute_value=True)
                        nc.vector.tensor_tensor(out=ssacc[:, m:m + 1],
                                                in0=ssacc[:, m:m + 1], in1=tmax[:],
                                                op=mybir.AluOpType.max)
                        nc.sync.dma_start(out=fsbuf[m * P:(m + 1) * P, noff:noff + nsz],
                                          in_=fsc[:, :nsz])

            # ---------------- Phase 2: gate + requant + transpose ----------------
            with tc.tile_pool(name="zbuf", bufs=2) as zp, \
                 tc.tile_pool(name="ystream", bufs=6) as yp, \
                 tc.tile_pool(name="p2tmp", bufs=4) as tp, \
                 tc.tile_pool(name="p2small", bufs=2) as sp, \
                 tc.tile_pool(name="p2stage", bufs=3) as stp, \
                 tc.tile_pool(name="psum2", bufs=3, space="PSUM") as pp2:
                for m in range(M_TILES):
                    zb1 = zp.tile([P, H], F32, tag="zb1", name=f"zb1_{m}")

                    inv2 = _inv_from_amax(nc, sp, s2acc[:, m:m + 1], "inv2")
                    s2t = sp.tile([P, 1], F32, tag="s2t")
                    nc.vector.tensor_scalar(s2t[:], s2acc[:, m:m + 1], sxt[:, m:m + 1],
                                            1.0 / QMAX, mybir.AluOpType.mult,
                                            mybir.AluOpType.mult)
                    inv_s = _inv_from_amax(nc, sp, ssacc[:, m:m + 1], "invs")
                    ss = sp.tile([P, 1], F32, tag="ss")
                    nc.vector.tensor_scalar(ss[:], ssacc[:, m:m + 1], s2t[:], 1.0 / QMAX,
                                            mybir.AluOpType.mult, mybir.AluOpType.mult)

                    # pass B: zb1 <- prod = (round(f_silu*inv_s)*ss) * round(z2*inv2)
                    mmax = sp.tile([P, 1], F32, tag="mmax")
                    nc.vector.memset(mmax[:], 0.0)
                    for (noff, nsz) in N1_CHUNKS:
                        c1 = zb1[:, noff:noff + nsz]
                        fsc = yp.tile([P, N1], F32, tag="fscB")
                        nc.sync.dma_start(out=fsc[:, :nsz],
                                          in_=fsbuf[m * P:(m + 1) * P, noff:noff + nsz])
                        y2c = yp.tile([P, N1], F32, tag="y2c")
                        nc.sync.dma_start(out=y2c[:, :nsz],
                                          in_=y2buf[m * P:(m + 1) * P, noff:noff + nsz])
                        c2 = y2c[:, :nsz]
                        tb = tp.tile([P, N1], F32, tag="tb")
                        nc.scalar.activation(out=tb[:, :nsz], in_=c2,
                                             func=mybir.ActivationFunctionType.Copy,
                                             bias=MAGIC, scale=inv2[:])
                        nc.vector.tensor_scalar(c2, tb[:, :nsz], MAGIC, None,
                                                mybir.AluOpType.subtract)
                        qs = tp.tile([P, N1], F32, tag="qs")
                        tq = tp.tile([P, N1], F32, tag="tq")
                        nc.scalar.activation(out=tq[:, :nsz], in_=fsc[:, :nsz],
                                             func=mybir.ActivationFunctionType.Copy,
                                             bias=MAGIC, scale=inv_s[:])
                        nc.vector.tensor_scalar(qs[:, :nsz], tq[:, :nsz], MAGIC, None,
                                                mybir.AluOpType.subtract)
                        nc.vector.scalar_tensor_tensor(out=c1, in0=qs[:, :nsz], scalar=ss[:],
                                                       in1=c2, op0=mybir.AluOpType.mult,
                                                       op1=mybir.AluOpType.mult)
                        tmax = tp.tile([P, 1], F32, tag="tmax2")
                        nc.vector.tensor_reduce(out=tmax[:], in_=c1,
                                                axis=mybir.AxisListType.X,
                                                op=mybir.AluOpType.max,
                                                apply_absolute_value=True)
                        nc.vector.tensor_tensor(out=mmax[:], in0=mmax[:], in1=tmax[:],
                                                op=mybir.AluOpType.max)

                    inv_m = _inv_from_amax(nc, sp, mmax[:], "invm")
                    nc.vector.tensor_scalar(smt[:, m:m + 1], mmax[:], 1.0 / QMAX, None,
                                            mybir.AluOpType.mult)

                    # pass C: q_mul = round(prod*inv_m); PE-transpose -> qmt
                    for ci, (noff, nsz) in enumerate(N1_CHUNKS):
                        c1 = zb1[:, noff:noff + nsz]
                        qm = tp.tile([P, N1], F32, tag="qm")
                        tm = tp.tile([P, N1], F32, tag="tm")
                        nc.scalar.activation(out=tm[:, :nsz], in_=c1,
                                             func=mybir.ActivationFunctionType.Copy,
                                             bias=MAGIC, scale=inv_m[:])
                        nc.vector.tensor_scalar(qm[:, :nsz], tm[:, :nsz], MAGIC, None,
                                                mybir.AluOpType.subtract)
                        nblk = nsz // P
                        stage = stp.tile([P, KSUB, P], BF16, tag="stage")
                        pst = pp2.tile([P, KSUB * P], F32, tag="pt")
                        for i in range(nblk):
                            nc.tensor.transpose(pst[:, i * P:(i + 1) * P],
                                                qm[:, i * P:(i + 1) * P], ident[:])
                        nc.scalar.copy(out=stage[:, :nblk, :].rearrange("p a b -> p (a b)"),
                                       in_=pst[:, :nblk * P])
                        nc.sync.dma_start(
                            out=qmt.ap()[ci * (N1 // P): ci * (N1 // P) + nblk, :,
                                         m * P:(m + 1) * P].rearrange("hb p t -> p hb t"),
                            in_=stage[:, :nblk, :])

            # ---------------- Phase 3: fc3 + final quant ----------------
            with tc.tile_pool(name="y3buf", bufs=1) as y3p, \
                 tc.tile_pool(name="qstream", bufs=10) as qsp, \
                 tc.tile_pool(name="w3stream", bufs=10) as w3p, \
                 tc.tile_pool(name="sw3rep", bufs=2) as sr3, \
                 tc.tile_pool(name="evac3", bufs=4) as e3, \
                 tc.tile_pool(name="outp", bufs=2) as op_, \
                 tc.tile_pool(name="psum3", bufs=1, space="PSUM") as pp3:
                y3b = y3p.tile([P, M_TILES, D], F32)
                for n3 in range(N3_CHUNKS):
                    swrep3 = sr3.tile([P, N3], F32, tag="sw3")
                    nc.sync.dma_start(
                        out=swrep3[:],
                        in_=s_w3.ap()[None, n3 * N3:(n3 + 1) * N3].to_broadcast([P, N3]))
                    ps_tiles = [pp3.tile([P, N3], F32, tag=f"p3_{m}", name=f"p3_{n3}_{m}")
                                for m in range(M_TILES)]
                    for k in range(K3):
                        qt = qsp.tile([P, TOK], BF16, tag="qmk")
                        nc.sync.dma_start(out=qt[:], in_=qmt.ap()[k])
                        wt3 = w3p.tile([P, N3], BF16, tag="w3t")
                        nc.sync.dma_start(out=wt3[:], in_=w3v[k, :, n3 * N3:(n3 + 1) * N3])
                        for m in range(M_TILES):
                            nc.tensor.matmul(ps_tiles[m][:], qt[:, m * P:(m + 1) * P],
                                             wt3[:], start=(k == 0), stop=(k == K3 - 1))
                    for m in range(M_TILES):
                        nc.vector.tensor_tensor(out=y3b[:, m, n3 * N3:(n3 + 1) * N3],
                                                in0=ps_tiles[m][:], in1=swrep3[:],
                                                op=mybir.AluOpType.mult)
                        tmax = e3.tile([P, 1], F32, tag="tmax3")
                        nc.vector.tensor_reduce(out=tmax[:],
                                                in_=y3b[:, m, n3 * N3:(n3 + 1) * N3],
                                                axis=mybir.AxisListType.X,
                                                op=mybir.AluOpType.max,
                                                apply_absolute_value=True)
                        nc.vector.tensor_tensor(out=amax3[:, m:m + 1],
                                                in0=amax3[:, m:m + 1], in1=tmax[:],
                                                op=mybir.AluOpType.max)
                for m in range(M_TILES):
                    inv3 = _inv_from_amax(nc, e3, amax3[:, m:m + 1], "inv3")
                    ost = op_.tile([P, D], I8, tag="ost")
                    for c in range(N3_CHUNKS):
                        t = e3.tile([P, N3], F32, tag="fq")
                        nc.scalar.activation(out=t[:], in_=y3b[:, m, c * N3:(c + 1) * N3],
                                             func=mybir.ActivationFunctionType.Copy,
                                             bias=MAGIC, scale=inv3[:])
                        nc.vector.tensor_scalar(ost[:, c * N3:(c + 1) * N3], t[:], MAGIC,
                                                None, mybir.AluOpType.subtract)
                    nc.sync.dma_start(out=out[m * P:(m + 1) * P, :], in_=ost[:])
                    so = e3.tile([P, 1], F32, tag="so")
                    nc.vector.tensor_scalar(so[:], amax3[:, m:m + 1], smt[:, m:m + 1],
                                            1.0 / QMAX, mybir.AluOpType.mult,
                                            mybir.AluOpType.mult)
                    nc.sync.dma_start(out=sov[m], in_=so[:, 0])
    nc.compile()
    return nc


def _get_nc():
    global _CACHED_NC
    if _CACHED_NC is None:
        _CACHED_NC = build_nc()
    return _CACHED_NC


def kernel(x, scale_x, w1, s_w1, w2, s_w2, w3, s_w3, _trace=False):
    nc = _get_nc()

    x = np.asarray(x, dtype=np.int8)
    scale_x = np.asarray(scale_x, dtype=np.float32)
    w1b = np.asarray(w1, dtype=np.int8).astype(ml_dtypes.bfloat16)
    w2b = np.asarray(w2, dtype=np.int8).astype(ml_dtypes.bfloat16)
    w3b = np.asarray(w3, dtype=np.int8).astype(ml_dtypes.bfloat16)
    s_w1 = np.ascontiguousarray(np.asarray(s_w1, dtype=np.float32))
    s_w2 = np.ascontiguousarray(np.asarray(s_w2, dtype=np.float32))
    s_w3 = np.ascontiguousarray(np.asarray(s_w3, dtype=np.float32))

    x_flat = x.reshape(B * S, D)
    sx_flat = scale_x.reshape(B * S)

    in_maps = []
    for c in range(NCORES):
        sl = slice(c * TOK, (c + 1) * TOK)
        xT = np.ascontiguousarray(x_flat[sl].T).astype(ml_dtypes.bfloat16)
        in_maps.append({
            "xT": xT,
            "w1": w1b, "w2": w2b, "w3": w3b,
            "scale_x": np.ascontiguousarray(sx_flat[sl]),
            "s_w1": s_w1, "s_w2": s_w2, "s_w3": s_w3,
        })

    res = run_bass_kernel_spmd(nc, in_maps, core_ids=list(range(NCORES)), trace=_trace)

    out = np.empty((B * S, D), dtype=np.int8)
    so = np.empty((B * S,), dtype=np.float32)
    for c in range(NCORES):
        sl = slice(c * TOK, (c + 1) * TOK)
        out[sl] = res.results[c]["out"]
        so[sl] = res.results[c]["scale_out"]
    if _trace:
        kernel.last_exec_time_ns = res.exec_time_ns
        kernel.last_results = res
    return out.reshape(B, S, D), so.reshape(B, S)
